# revision 15
# baseline (speedup 1.0000x reference)
"""Trainium2 Bass kernel for a 2-layer cosine-similarity attention GCN.

Reference math (per (b,h) slice, two chained blocks):
    xn = x / max(||x||_row, eps)
    A  = softmax((xn @ xn^T) / max(alpha, 0.01), axis=-1)
    out = relu((A @ x) @ W^T + x)

Shapes: x [4, 4, 4096, 64] fp32; W [64, 64]. B*H = 16 slices sharded as
2 slices per NeuronCore across 8 cores (fully independent, no collectives).

Kernel strategy (per core, 2 pairs x 2 blocks, all on-chip):
  - softmax without max-subtraction (logits are bounded cosine sims):
    P = exp(S*scale)/Z, Z = rowsum via a ones column in the AV matmul.
  - W fused into the AV matmul: lhsT = [1 | x@W^T | 0pad] so U = [Z; G].
  - exp split between the scalar engine (true exp) and the vector engine
    (Schraudolph bit-trick exp emitted directly as fp16 bits, 2 ops).
  - flattened software-pipelined step loop over all (chunk, tile) steps:
    S matmuls (N=1024, the two pairs in different PE row groups), exp, AV
    matmuls at lag 2; chunk epilogues and next-block prep are sliced into
    small deferred pieces drained a few per step so no engine ever sees a
    multi-us bubble (keeps the PE HAM clock-gate at 8/8).
  - U ([80, CHW] psum) -> bf16 -> DMA-xbar transpose to row-major; divide
    by Z / residual / relu done row-major where Z is a per-partition scalar.
"""

import numpy as np

import concourse.bacc as bacc
import concourse.tile as tile
from concourse import mybir
from concourse.bass_utils import run_bass_kernel_spmd
from concourse.masks import make_identity
from concourse.dve_ops import TENSOR_TENSOR_REDUCE

F32 = mybir.dt.float32
FP16 = mybir.dt.float16
I32 = mybir.dt.int32
U16 = mybir.dt.uint16
BF16 = mybir.dt.bfloat16
AF = mybir.ActivationFunctionType
ALU = mybir.AluOpType

P = 128
D = 64
N_CORES = 8
ACT_W = 704          # columns of each 1024-chunk exp'd on the scalar engine
MAGIC = 0x5F3759DF   # fast inverse sqrt seed


def _schraudolph_consts(scale):
    """Constants for the 1-op DVE exp producing fp16 bits directly:
    fp16bits(exp(scale*s)) ~= u16(round(s*scale*K16 + BIAS)) -- the arith
    tensor_scalar's fp32->u16 writeback conversion does the float->int."""
    K16 = 2.0 ** 10 / np.log(2.0)
    c_err = 44
    return float(scale * K16), float(15 * 1024 - c_err)


def build_nc(scales, n_rows=4096, npairs=2):
    nblocks = len(scales)
    NT = n_rows // P             # 128-row tiles per pair
    CHW = 1024                   # i-chunk width
    NCH = n_rows // CHW
    GPC = CHW // P               # row-major gi tiles per chunk (8)
    DVE_W = CHW - ACT_W
    MW = 80                      # U partitions: [Z | G(64) | pad(15)]
    LAG = 2                      # steps between S/exp production and AV use

    nc = bacc.Bacc("TRN2", target_bir_lowering=False, debug=False, num_devices=N_CORES)
    xin = nc.dram_tensor("xin", [npairs, n_rows, D], F32, kind="ExternalInput").ap()
    wts = [
        nc.dram_tensor(f"w{i}t", [D, D], F32, kind="ExternalInput").ap()
        for i in range(nblocks)
    ]
    out = nc.dram_tensor("out", [npairs, n_rows, D], F32, kind="ExternalOutput").ap()

    xin_t = xin.rearrange("p (t pp) d -> p pp t d", pp=P)  # [np, 128, NT, 64]
    out_t = out.rearrange("p (t pp) d -> p pp t d", pp=P)

    with tile.TileContext(nc) as tc:
        with (
            tc.tile_pool(name="singles", bufs=1) as singles,
            tc.tile_pool(name="stats", bufs=2) as stats,
            tc.tile_pool(name="tmp", bufs=3) as tmp,
            tc.tile_pool(name="epool", bufs=6) as epool,
            tc.tile_pool(name="fin", bufs=2) as fin,
            tc.tile_pool(name="ps_s", bufs=2, space="PSUM") as ps_s,
            tc.tile_pool(name="ps_u", bufs=2, space="PSUM") as ps_u,
        ):
            ident16 = singles.tile([P, P], BF16, tag="ident16")
            make_identity(nc, ident16[:])

            # PE warm-up: ~5us of back-to-back matmuls during the input DMA
            # window so the HAM clock-gate reaches 8/8 before processing.
            wup = ps_s.tile([P, P], F32, tag="S", name="wup")
            for _ in range(40):
                nc.tensor.matmul(
                    wup[:], lhsT=ident16[:], rhs=ident16[:], start=True, stop=True
                )

            # W.T tiles, cast to bf16, replicated on both partition halves so
            # pair-1 xw matmuls (lhsT based at partition 64) see them too.
            wt16 = []
            for i in range(nblocks):
                wf = singles.tile([D, D], F32, tag=f"wf{i}", name=f"wf{i}")
                nc.sync.dma_start(wf[:], wts[i])
                w16 = singles.tile([P, D], BF16, tag=f"w16_{i}", name=f"w16_{i}")
                nc.vector.tensor_copy(w16[0:D, :], wf[:])
                nc.vector.tensor_copy(w16[D:P, :], wf[:])
                wt16.append(w16)

            # Persistent per (pair, block) state.
            xnt = {}   # normalized rows, transposed: pair p at partitions [64p, 64p+64)
            xb = {}    # block input, row-major fp32 [128, NT, 64]
            xw16 = {}  # AV-matmul stationary operand: [1 | x@W^T | 0] fp16
            for blk in range(nblocks):
                xnt[blk] = singles.tile([P, n_rows], BF16, tag=f"xnt_{blk}", name=f"xnt_{blk}")
            for p in range(npairs):
                for blk in range(nblocks):
                    xb[p, blk] = singles.tile(
                        [P, NT, D], F32, tag=f"xb_{p}_{blk}", name=f"xb_{p}_{blk}"
                    )
                    xw16[p, blk] = singles.tile(
                        [P, NT, MW], FP16, tag=f"xw_{p}_{blk}", name=f"xw_{p}_{blk}"
                    )
                    nc.vector.memset(xw16[p, blk][:], 0.0)
                    nc.vector.memset(xw16[p, blk][:, :, 0:1], 1.0)

            # norms^2 / 1/norm / norm accumulators per (pair, block)
            s_all = {}
            rin_all = {}
            nrm_all = {}
            for p in range(npairs):
                for blk in range(nblocks):
                    s_all[p, blk] = singles.tile(
                        [P, NT], F32, tag=f"sall_{p}_{blk}", name=f"sall_{p}_{blk}"
                    )
                    rin_all[p, blk] = singles.tile(
                        [P, NT], F32, tag=f"rin_{p}_{blk}", name=f"rin_{p}_{blk}"
                    )
                    nrm_all[p, blk] = singles.tile(
                        [P, NT], F32, tag=f"nrm_{p}_{blk}", name=f"nrm_{p}_{blk}"
                    )

            for p in range(npairs):
                for c in range(NCH):
                    g0 = c * (NT // NCH)
                    g1 = (c + 1) * (NT // NCH)
                    nc.sync.dma_start(xb[p, 0][:, g0:g1, :], xin_t[p][:, g0:g1, :])

            def rsqrt_slice(p, blk, c0, n):
                """rin = s^-0.5, nrm = s*rin for columns [c0, c0+n)."""
                s_t = s_all[p, blk][:, c0 : c0 + n]
                r_t = rin_all[p, blk][:, c0 : c0 + n]
                nc.vector.tensor_scalar_max(s_t, s_t, 1e-24)
                s_i = s_t.bitcast(I32)
                r_i = r_t.bitcast(I32)
                nc.vector.tensor_scalar(
                    out=r_i, in0=s_i, scalar1=1, scalar2=None,
                    op0=ALU.logical_shift_right,
                )
                nc.vector.tensor_scalar(
                    out=r_i, in0=r_i, scalar1=MAGIC, scalar2=None, op0=ALU.subtract,
                )
                nc.vector.tensor_scalar(
                    out=r_i, in0=r_i, scalar1=-1, scalar2=None, op0=ALU.bitwise_xor,
                )
                nc.vector.tensor_scalar(
                    out=r_i, in0=r_i, scalar1=1, scalar2=None, op0=ALU.add,
                )
                t1 = stats.tile([P, NT], F32, tag="nt1")
                t1v = t1[:, 0:n]
                for _ in range(3):
                    nc.vector.tensor_mul(t1v, r_t, r_t)
                    nc.vector.tensor_mul(t1v, t1v, s_t)
                    nc.vector.tensor_scalar(
                        out=t1v, in0=t1v, scalar1=-0.5, scalar2=1.5,
                        op0=ALU.mult, op1=ALU.add,
                    )
                    nc.vector.tensor_mul(r_t, r_t, t1v)
                nc.vector.tensor_mul(nrm_all[p, blk][:, c0 : c0 + n], s_t, r_t)

            def xn_tile(p, blk, b):
                """normalize tile b, PE-transpose it into xnt."""
                lo = D * p
                xn16 = tmp.tile([P, D], BF16, tag="xn16")
                nc.vector.tensor_scalar_mul(
                    xn16[:], xb[p, blk][:, b, :], rin_all[p, blk][:, b : b + 1]
                )
                pst = ps_s.tile([P, P], BF16, tag="S")
                nc.tensor.transpose(pst[lo : lo + D, :], xn16[:], ident16[:])
                nc.vector.tensor_copy(
                    xnt[blk][lo : lo + D, b * P : (b + 1) * P], pst[lo : lo + D, :]
                )

            def xw_tile(p, blk, b):
                """xw16[:, b, 1:65] = norm_b * (xn_b @ W^T) via matmul + scale."""
                lo = D * p
                psw = ps_s.tile([P, D], F32, tag="S")
                nc.tensor.matmul(
                    psw[:],
                    lhsT=xnt[blk][lo : lo + D, b * P : (b + 1) * P],
                    rhs=wt16[blk][lo : lo + D, :],
                    start=True, stop=True,
                )
                nc.vector.tensor_scalar_mul(
                    xw16[p, blk][:, b, 1 : 1 + D], psw[:], nrm_all[p, blk][:, b : b + 1]
                )

            def prep0(p):
                """Upfront prep for block 0, chunked so the PE starts early."""
                sq = tmp.tile([P, D], F32, tag="sq")
                gpc0 = NT // NCH
                for c in range(NCH):
                    for b in range(c * gpc0, (c + 1) * gpc0):
                        nc.scalar.activation(
                            sq[:], xb[p, 0][:, b, :], AF.Square,
                            accum_out=s_all[p, 0][:, b : b + 1],
                        )
                    rsqrt_slice(p, 0, c * gpc0, gpc0)
                    for b in range(c * gpc0, (c + 1) * gpc0):
                        xn_tile(p, 0, b)
                for b in range(NT):
                    xw_tile(p, 0, b)

            def process(blk, scale, last, deferred):
                smul, sbias = _schraudolph_consts(scale)
                nsteps = NCH * NT
                U = {}        # chunk -> pair -> psum tile
                E_hist = {}   # step -> pair -> E tile

                def u_mms(k):
                    a_, b_ = divmod(k, NT)
                    for p in range(npairs):
                        for h in range(2):
                            nc.tensor.matmul(
                                U[a_][p][:, h * 512 : (h + 1) * 512],
                                lhsT=xw16[p, blk][:, b_, :],
                                rhs=E_hist[k][p][:, h * 512 : (h + 1) * 512],
                                start=(b_ == 0),
                                stop=(b_ == NT - 1),
                            )
                    if k in E_hist:
                        del E_hist[k]

                def make_epilogue_pieces(a):
                    """Small deferred closures; drained a few per step."""
                    pieces = []
                    gi0 = a * GPC
                    u16 = {}
                    T = {}
                    rz = {}
                    gm = {}

                    def mk(p):
                        def c_copy_l():
                            u16[p] = fin.tile([MW, CHW], BF16, tag="u16", name=f"u16_{blk}_{a}_{p}")
                            nc.vector.tensor_copy(u16[p][:, 0:512], U[a][p][:, 0:512])

                        def c_copy_r():
                            nc.vector.tensor_copy(u16[p][:, 512:1024], U[a][p][:, 512:1024])

                        def c_tr():
                            T[p] = fin.tile([P, GPC, MW], BF16, tag="T", name=f"T_{blk}_{a}_{p}")
                            nc.sync.dma_start_transpose(T[p][:], u16[p][:])
                            rz[p] = tmp.tile([P, GPC], F32, tag="rz", name=f"rz_{blk}_{a}_{p}")
                            nc.vector.reciprocal(rz[p][:], T[p][:, :, 0])

                        def c_mul_lo():
                            gm[p] = fin.tile([P, GPC, D], F32, tag="gm", name=f"gm_{blk}_{a}_{p}")
                            for t in range(GPC // 2):
                                nc.vector.tensor_scalar_mul(
                                    gm[p][:, t, :], T[p][:, t, 1 : 1 + D],
                                    rz[p][:, t : t + 1],
                                )

                        def c_mul_hi():
                            for t in range(GPC // 2, GPC):
                                nc.vector.tensor_scalar_mul(
                                    gm[p][:, t, :], T[p][:, t, 1 : 1 + D],
                                    rz[p][:, t : t + 1],
                                )

                        def c_add():
                            nc.vector.tensor_add(
                                gm[p][:], gm[p][:], xb[p, blk][:, gi0 : gi0 + GPC, :]
                            )

                        def c_relu():
                            if not last:
                                dst = xb[p, blk + 1][:, gi0 : gi0 + GPC, :]
                                nc.vector.tensor_scalar_max(dst, gm[p][:], 0.0)
                            else:
                                oo = fin.tile([P, GPC, D], F32, tag="oo", name=f"oo_{blk}_{a}_{p}")
                                nc.vector.tensor_scalar_max(oo[:], gm[p][:], 0.0)
                                nc.sync.dma_start(out_t[p][:, gi0 : gi0 + GPC, :], oo[:])

                        return [c_copy_l, c_copy_r, c_tr, c_mul_lo, c_mul_hi, c_add, c_relu]

                    per_p = [mk(p) for p in range(npairs)]
                    for idx in range(len(per_p[0])):
                        for p in range(npairs):
                            pieces.append(per_p[p][idx])

                    if not last:
                        # next block's norms/rsqrt/xnt/xw for this chunk's tiles
                        def mk_prep(p):
                            sub = []

                            def c_norms_lo():
                                sqo = tmp.tile([P, D], F32, tag="sqo")
                                for t in range(GPC // 2):
                                    gi = gi0 + t
                                    nc.vector._custom_dve(
                                        TENSOR_TENSOR_REDUCE,
                                        out=sqo[:],
                                        in0=xb[p, blk + 1][:, gi, :],
                                        in1=xb[p, blk + 1][:, gi, :],
                                        s0=0.0, s1=1.0,
                                        accum_out=s_all[p, blk + 1][:, gi : gi + 1],
                                    )

                            def c_norms_hi():
                                sqo = tmp.tile([P, D], F32, tag="sqo")
                                for t in range(GPC // 2, GPC):
                                    gi = gi0 + t
                                    nc.vector._custom_dve(
                                        TENSOR_TENSOR_REDUCE,
                                        out=sqo[:],
                                        in0=xb[p, blk + 1][:, gi, :],
                                        in1=xb[p, blk + 1][:, gi, :],
                                        s0=0.0, s1=1.0,
                                        accum_out=s_all[p, blk + 1][:, gi : gi + 1],
                                    )

                            def c_rsqrt():
                                rsqrt_slice(p, blk + 1, gi0, GPC)

                            sub.extend([c_norms_lo, c_norms_hi, c_rsqrt])
                            for t in range(GPC):
                                sub.append(
                                    (lambda tt: lambda: xn_tile(p, blk + 1, gi0 + tt))(t)
                                )
                            for t0 in range(0, GPC, 4):
                                def c_xw(p=p, t0=t0):
                                    for t in range(t0, t0 + 4):
                                        xw_tile(p, blk + 1, gi0 + t)
                                sub.append(c_xw)
                            return sub

                        per_pp = [mk_prep(p) for p in range(npairs)]
                        for idx in range(len(per_pp[0])):
                            for p in range(npairs):
                                pieces.append(per_pp[p][idx])
                    return pieces

                for k in range(nsteps):
                    a, b = divmod(k, NT)
                    if b == 0:
                        U[a] = {
                            p: ps_u.tile([MW, CHW], F32, tag="U", name=f"U_{blk}_{a}_{p}")
                            for p in range(npairs)
                        }
                    # AV matmuls at lag (issued first so the S pair stays adjacent)
                    if k >= LAG:
                        u_mms(k - LAG)
                        kk = k - LAG
                        if kk % NT == NT - 1:
                            deferred.extend(make_epilogue_pieces(kk // NT))
                    # S matmuls, both pairs adjacent (different PE row groups)
                    S = {}
                    for p in range(npairs):
                        S[p] = ps_s.tile([P, CHW], F32, tag="S", name=f"S_{k}_{p}")
                    for h in range(2):
                        for p in range(npairs):
                            lo = D * p
                            nc.tensor.matmul(
                                S[p][:, h * 512 : (h + 1) * 512],
                                lhsT=xnt[blk][lo : lo + D, b * P : (b + 1) * P],
                                rhs=xnt[blk][
                                    lo : lo + D, a * CHW + h * 512 : a * CHW + (h + 1) * 512
                                ],
                                start=True, stop=True,
                            )
                    # exp: scalar engine + vector engine split
                    E_hist[k] = {}
                    for p in range(npairs):
                        Ek = epool.tile([P, CHW], FP16, tag="E", name=f"E_{k}_{p}")
                        E_hist[k][p] = Ek
                        nc.scalar.activation(
                            Ek[:, 0:ACT_W], S[p][:, 0:ACT_W], AF.Exp, scale=scale
                        )
                        nc.vector.tensor_scalar(
                            out=Ek[:, ACT_W:CHW].bitcast(U16),
                            in0=S[p][:, ACT_W:CHW],
                            scalar1=smul, scalar2=sbias,
                            op0=ALU.mult, op1=ALU.add,
                        )
                    # drain deferred pieces
                    for _ in range(3):
                        if deferred:
                            deferred.pop(0)()

                # tail: remaining AV matmuls + last chunk's epilogue enqueued;
                # the deferred queue carries over into the next block.
                for k in range(nsteps - LAG, nsteps):
                    u_mms(k)
                    kk = k
                    if kk % NT == NT - 1:
                        deferred.extend(make_epilogue_pieces(kk // NT))

            for p in range(npairs):
                prep0(p)
            deferred = []
            for blk in range(nblocks):
                process(blk, scales[blk], last=(blk == nblocks - 1), deferred=deferred)
            while deferred:
                deferred.pop(0)()

    nc.compile()
    return nc


_CACHE = {}


def _get_nc(scales, n_rows, npairs):
    key = (tuple(scales), n_rows, npairs)
    if key not in _CACHE:
        _CACHE[key] = build_nc(list(scales), n_rows=n_rows, npairs=npairs)
    return _CACHE[key]


def kernel(x, W1, W2, alpha1, alpha2):
    x = np.asarray(x, dtype=np.float32)
    B, H, N, d = x.shape
    assert d == D and (B * H) % N_CORES == 0
    npairs = (B * H) // N_CORES
    s1 = 1.0 / max(float(alpha1), 0.01)
    s2 = 1.0 / max(float(alpha2), 0.01)
    nc = _get_nc((s1, s2), N, npairs)

    xf = np.ascontiguousarray(x.reshape(B * H, N, d))
    w0 = np.ascontiguousarray(np.asarray(W1, dtype=np.float32).T)
    w1 = np.ascontiguousarray(np.asarray(W2, dtype=np.float32).T)
    in_maps = [
        {"xin": xf[npairs * c : npairs * (c + 1)], "w0t": w0, "w1t": w1}
        for c in range(N_CORES)
    ]
    res = run_bass_kernel_spmd(nc, in_maps, core_ids=list(range(N_CORES)))
    outs = np.stack([r["out"] for r in res.results])
    return outs.reshape(B, H, N, d).astype(np.float32)
